# revision 61
# baseline (speedup 1.0000x reference)
"""Causal self-attention for Trainium2, 8 NeuronCores.

Sharding: tensor-parallel over heads (4 heads/core) x data-parallel over
batch (2). Core i handles batch i//4, heads 4*(i%4)..4*(i%4)+3. Each core
computes its heads' attention output and a partial output projection
(W_proj rows for its heads); the host sums the 4 partials per batch and
adds b_proj.

Device layout choices:
  - Q^T, K^T computed feature-major [dim, t] directly (lhsT = W chunk,
    rhs = x^T chunk), so attention scores come out as S^T [k, q] with k
    on partitions -- which is exactly the layout the P@V matmul needs
    as its rhs. No on-chip transposes of the O(T^2) object.
  - V computed in natural [t, dim] layout (lhsT = x^T chunk, rhs = W_v),
    which is the lhsT layout the P@V matmul needs. A ones-column is
    appended to V so the softmax denominators fall out of the same
    matmul (PSUM partition 64 of each head's accumulator).
  - exp() without max subtraction: scores are q.k/8 with q,k ~ N(0,1),
    bounded well inside fp32 exp range; softmax is shift-invariant so
    the result is mathematically identical to the reference.

Causal handling: fully-masked k-chunks are skipped; on diagonal chunks
the S^T and P@V matmuls are column-trimmed to the unmasked q-range
(PSUM zero-regions are bank-row sized, so a full-width start followed
by narrower accumulates is legal), and the 128-wide triangular block
is zeroed after the exp by one shared 0/1 indicator tile. If the
runtime mask is not the lower-tri causal mask, a general fallback
multiplies p by the actual mask (DMA'd transposed) after the exp; an
all-ones mask drops masking entirely.

Engine budget (the PE is the bottleneck at ~124us busy; everything else
is placed to never stall it): PE does matmuls only; ACT does the exp
chain (~75us, the co-bottleneck in the last q-block) plus a couple of
tail copies; DVE does all PSUM-reading elementwise work (QK/V bias
casts, stash casts, projection casts, diag masks, reciprocals) -- the
Pool/GpSimd engine does only reciprocal partition-broadcasts and DMA
dispatch, because on TRN2 it cannot touch PSUM and its tensor ops run
on slow Q7 DSP cores.

Softmax denominators: V's ones-column puts sum(p) on PSUM partition 64
of each head's P@V accumulator; plain -64-partition-shifted vector
copies pull the two rows to partition 0 (custom-DVE reciprocal and
partition_broadcast silently mis-read non-zero partition bases on HW,
and a gpsimd DMA here costs ~4us of SWDGE latency that freezes the
in-order vector queue), then reciprocal + gpsimd broadcast + in-place
OT scale, all deferred a few units so nothing waits. The tail pair
instead broadcasts via two one-row PE matmuls (contraction 1, all
operands at partition 0) so the gpsimd queue never gates the drain,
while pre-started dq0 projection matmuls keep the PE's p-state hot
(TRN2's PE clock ramps 0.65->1.2->2.4GHz with ~3us of continuous
execution and resets on every idle gap -- sustained feed beats an
early trickle, which is also why the startup DMA is batched in pairs).

Scheduling: the output projection is backloaded (2/4/6 tiles injected
into blocks qc1/qc2/qc3) because qc3's exp work (~27us) far exceeds
its attention matmul work (~12us) -- the spare projection tiles keep
the PE busy while ACT grinds through the last exps.
"""

import numpy as np

B, T, C, H = 2, 2048, 1024, 16
D = C // H            # 64 head dim
NCORES = 8
NBG = 2               # batch shards
NHG = 4               # head-group shards
HL = H // NHG         # 4 heads per core
DL = HL * D           # 256 local feature dims
NDQ = DL // 128       # 2 partition chunks of local dims
NTB = T // 512        # 4 t-chunks of 512
NKC = T // 128        # 16 key chunks of 128
NQC = T // 512        # 4 query chunks of 512
NTT = T // 128        # 16 t-tiles of 128 (proj / V)

_CACHE = {}


def _build(mode, debug_dump=False):
    """Build + compile the per-core Bass program. mode: causal|full|general."""
    import concourse.bass as bass
    import concourse.bacc as bacc
    import concourse.tile as tile
    import concourse.mybir as mybir

    f32 = mybir.dt.float32
    bf16 = mybir.dt.bfloat16
    Exp = mybir.ActivationFunctionType.Exp
    mult = mybir.AluOpType.mult
    add = mybir.AluOpType.add

    nc = bacc.Bacc(
        "TRN2", target_bir_lowering=False, debug=False, num_devices=NCORES
    )

    xT = nc.dram_tensor("xT", [C, T], bf16, kind="ExternalInput").ap()
    # W pre-permuted on the host so each column group is contiguous per
    # partition (2KB+ DMA lines; the naive [C, 3*DL] layout gives 256B
    # strided segments and ~5us group loads)
    WgQK = nc.dram_tensor(
        "WgQK", [128, 4, 8, 128], bf16, kind="ExternalInput"
    ).ap()
    WgV = nc.dram_tensor("WgV", [128, 8, 256], bf16, kind="ExternalInput").ap()
    # x's first t-chunk, host-permuted contiguous per partition (8KB DMA
    # lines): the startup is DMA-bound, later t-chunks stream fine from xT
    x0g = nc.dram_tensor("x0g", [128, 8, 512], bf16, kind="ExternalInput").ap()
    bqk = nc.dram_tensor("bqk", [128, 2 * NDQ], f32, kind="ExternalInput").ap()
    bv = nc.dram_tensor("bv", [1, DL], f32, kind="ExternalInput").ap()
    Wp = nc.dram_tensor("Wp", [DL, C], bf16, kind="ExternalInput").ap()
    maskT = None
    if mode == "general":
        maskT = nc.dram_tensor("maskT", [T, T], bf16, kind="ExternalInput").ap()
    yp = nc.dram_tensor("yp", [T, C], bf16, kind="ExternalOutput").ap()
    dbg = {}
    if debug_dump:
        for nm, shp, dt in [
            ("ot_d", [128, NDQ, T], bf16),
        ]:
            dbg[nm] = nc.dram_tensor(nm, shp, dt, kind="ExternalOutput").ap()

    with tile.TileContext(nc) as tc:
        with (
            tc.tile_pool(name="singles", bufs=1) as singles,
            tc.tile_pool(name="xin", bufs=2) as xin,
            tc.tile_pool(name="ptiles", bufs=8) as ptiles,
            tc.tile_pool(name="small", bufs=4) as small,
            tc.tile_pool(name="outp", bufs=4) as outp,
            tc.tile_pool(name="psum", bufs=7, space="PSUM") as psum,
        ):
            def ps512(name):
                # transient accumulators (qk/v/proj): 2-bank rotation
                return psum.tile(
                    [128, 512], f32, name="ps512", tag="ps512", bufs=2
                )

            def ps_ops(name):
                # attention P@V accumulators live in their own 2-bank tag:
                # they are held for a whole head-pair, and sharing a
                # rotation with the transients chained every third
                # injected unit onto a pair-long hold
                return psum.tile(
                    [128, 512], f32, name="ops", tag="ops", bufs=2
                )

            # ---- resident inputs ----
            # W loads by COLUMN GROUP in unit-consumption order (Q0, Q1,
            # K0, K1, V) on the sync queue; x's first t-chunk rides the
            # scalar (ACT) hwdge queue -- idle until the first exp -- so
            # the two dispatch chains run in parallel and the first QK
            # unit's operands land ~2us after the preamble.
            x0 = xin.tile([128, 8, 512], bf16, tag="x0", bufs=1)
            # three parallel dispatch chains: W column groups + first x
            # pairs on sync (both host-permuted contiguous per partition),
            # the rest of x then the small inputs on scalar
            Wq0 = singles.tile([128, 8, 128], bf16, name="Wq0")
            nc.sync.dma_start(out=x0[:, 0:2, :], in_=x0g[:, 0:2, :])
            nc.sync.dma_start(out=Wq0[:, 0:3, :], in_=WgQK[:, 0, 0:3, :])
            nc.sync.dma_start(out=x0[:, 2:4, :], in_=x0g[:, 2:4, :])
            nc.sync.dma_start(out=Wq0[:, 3:8, :], in_=WgQK[:, 0, 3:8, :])
            nc.scalar.dma_start(out=x0[:, 4:6, :], in_=x0g[:, 4:6, :])
            nc.scalar.dma_start(out=x0[:, 6:8, :], in_=x0g[:, 6:8, :])
            bqk_sb = singles.tile([128, 2 * NDQ], f32)
            nc.scalar.dma_start(out=bqk_sb, in_=bqk)
            bv_row = singles.tile([1, DL], f32)
            nc.scalar.dma_start(out=bv_row, in_=bv)
            Wcg = [Wq0]
            for gi in range(1, 4):
                wt = singles.tile([128, 8, 128], bf16, name=f"Wcg{gi}")
                nc.sync.dma_start(out=wt, in_=WgQK[:, gi, :, :])
                Wcg.append(wt)
            WV = singles.tile([128, 8, 256], bf16, name="WV")
            nc.sync.dma_start(out=WV, in_=WgV)

            bv_sb = singles.tile([128, DL], f32)
            nc.gpsimd.partition_broadcast(bv_sb, bv_row)

            # 0/1 indicator for the 128-wide diagonal block: every diag
            # chunk uses the same within-block mask (keep iff c >= p)
            indb = None
            if mode == "causal":
                indb = singles.tile([128, 128], bf16, name="indb")
                nc.vector.memset(indb, 1.0)
                nc.gpsimd.affine_select(
                    out=indb, in_=indb,
                    compare_op=mybir.AluOpType.is_ge,
                    fill=0.0, base=0, pattern=[[1, 128]],
                    channel_multiplier=-1,
                )

            # ones row for the tail's one-row broadcast matmuls
            ones1 = singles.tile([1, 64], bf16, name="ones1")
            nc.vector.memset(ones1, 1.0)

            # ---- resident intermediates ----
            QT = singles.tile([128, NDQ, T], bf16)   # [dim%128, dimchunk, t]
            KT = singles.tile([128, NDQ, T], bf16)
            # V plus a ones-column: each head uses cols [0:D+1], so its
            # softmax denominator lands on PSUM partition 64.
            V1 = singles.tile([128, NKC, HL, D + 1], bf16)
            nc.vector.memset(V1[:, :, :, D : D + 1], 1.0)
            OT = singles.tile([128, NDQ, T], bf16)
            Wp_sb = singles.tile([128, NDQ, C], bf16)

            # ---- phase 1: QKV projections (as interleavable units) ----
            def p1_units(tb, xs):
                """Units for one 512-wide t-chunk of the QKV projection.
                xs(kc) -> [128, 512] AP for contraction chunk kc."""
                units = []
                for s in range(2):  # 0=Q, 1=K
                    for dq in range(NDQ):
                        def qk_u(tb=tb, s=s, dq=dq, xs=xs):
                            ps = ps512("qk")
                            g = s * NDQ + dq
                            for kc in range(8):
                                nc.tensor.matmul(
                                    ps,
                                    lhsT=Wcg[g][:, kc, :],
                                    rhs=xs(kc),
                                    start=(kc == 0),
                                    stop=(kc == 7),
                                )
                            dst = (QT if s == 0 else KT)[
                                :, dq, tb * 512 : (tb + 1) * 512
                            ]
                            nc.vector.tensor_scalar_add(
                                dst, ps,
                                bqk_sb[:, g : g + 1],
                            )
                        units.append(qk_u)
                for t4 in range(4):
                    def v_u(tb=tb, t4=t4, xs=xs):
                        tt = tb * 4 + t4
                        ps = ps512("v")
                        for kc in range(8):
                            nc.tensor.matmul(
                                ps[:, :DL],
                                lhsT=xs(kc)[:, t4 * 128 : (t4 + 1) * 128],
                                rhs=WV[:, kc, :],
                                start=(kc == 0),
                                stop=(kc == 7),
                            )
                        nc.vector.tensor_tensor(
                            out=V1[:, tt, :, 0:D],
                            in0=ps[:, :DL].rearrange("p (h d) -> p h d", d=D),
                            in1=bv_sb.rearrange("p (h d) -> p h d", d=D),
                            op=add,
                        )
                    units.append(v_u)
                return units

            # ---- projection units ----
            def proj_units(qc, tail=False, use_scalar=False):
                units = []
                for t4 in range(4):
                    def u(qc=qc, t4=t4, tail=tail):
                        tt = qc * 4 + t4
                        y_sb = outp.tile([128, C], bf16, name="y_sb")
                        for n in range(2):
                            pp = ps512("proj")
                            for dq in range(NDQ):
                                nc.tensor.matmul(
                                    pp,
                                    lhsT=OT[:, dq, tt * 128 : (tt + 1) * 128],
                                    rhs=Wp_sb[:, dq, n * 512 : (n + 1) * 512],
                                    start=(dq == 0),
                                    stop=(dq == NDQ - 1),
                                )
                            dst = y_sb[:, n * 512 : (n + 1) * 512]
                            if (tail or use_scalar) and n == 1:
                                nc.scalar.copy(dst, pp)
                            else:
                                nc.vector.tensor_copy(dst, pp)
                        if tail:
                            for n in range(2):
                                nc.sync.dma_start(
                                    out=yp[tt * 128 : (tt + 1) * 128,
                                           n * 512 : (n + 1) * 512],
                                    in_=y_sb[:, n * 512 : (n + 1) * 512],
                                )
                        else:
                            nc.sync.dma_start(
                                out=yp[tt * 128 : (tt + 1) * 128, :], in_=y_sb
                            )
                    units.append(u)
                return units

            # tail-only split projection: dq=0 accumulation pre-started
            # while the last pair's reciprocal/broadcast chain runs
            pp_state = {}

            def proj_head(tt, big=False, use_ops=False):
                if big:
                    tb = psum.tile(
                        [128, 2, 512], f32, name="ps1024", tag="ps1024",
                        bufs=2,
                    )
                for n in range(2):
                    pp = tb[:, n, :] if big else (
                        ps_ops("projA") if use_ops else ps512("projA")
                    )
                    nc.tensor.matmul(
                        pp,
                        lhsT=OT[:, 0, tt * 128 : (tt + 1) * 128],
                        rhs=Wp_sb[:, 0, n * 512 : (n + 1) * 512],
                        start=True,
                        stop=False,
                    )
                    pp_state[(tt, n)] = pp

            def proj_finish(tt):
                y_sb = outp.tile([128, C], bf16, name="y_sb")
                for n in range(2):
                    pp = pp_state.pop((tt, n))
                    nc.tensor.matmul(
                        pp,
                        lhsT=OT[:, 1, tt * 128 : (tt + 1) * 128],
                        rhs=Wp_sb[:, 1, n * 512 : (n + 1) * 512],
                        start=False,
                        stop=True,
                    )
                    dst = y_sb[:, n * 512 : (n + 1) * 512]
                    if n == 1:
                        nc.scalar.copy(dst, pp)
                    else:
                        nc.vector.tensor_copy(dst, pp)
                for n in range(2):
                    nc.sync.dma_start(
                        out=yp[tt * 128 : (tt + 1) * 128,
                               n * 512 : (n + 1) * 512],
                        in_=y_sb[:, n * 512 : (n + 1) * 512],
                    )

            norm_state = {}

            def attn_units(qc, hp, nkc, m_sb):
                """One head-pair's attention over all k-chunks; softmax
                denominators ride V's ones-column onto PSUM partition 64."""
                state = {}
                last = qc == NQC - 1 and hp == NDQ - 1

                def lo_of(kc):
                    if mode == "causal" and kc >= 4 * qc:
                        return 128 * (kc - 4 * qc)
                    return 0

                def emit_mm1(j):
                    lo = lo_of(j)
                    stp = psum.tile(
                        [128, 2, 512], f32, name="ps1024", tag="ps1024", bufs=2
                    )
                    for hh in range(2):
                        off = 64 * hh
                        nc.tensor.matmul(
                            stp[:, hh, lo:],
                            lhsT=KT[off : off + 64, hp, j * 128 : (j + 1) * 128],
                            rhs=QT[
                                off : off + 64, hp,
                                qc * 512 + lo : (qc + 1) * 512,
                            ],
                            start=True,
                            stop=True,
                        )
                    state.setdefault("st", {})[j] = stp

                def consume(kc):
                    if kc == 0:
                        state["ops"] = [ps_ops("o"), ps_ops("o")]
                    ops = state["ops"]
                    stp = state["st"].pop(kc)
                    lo = lo_of(kc)
                    diag = mode == "causal" and kc >= 4 * qc
                    p2 = ptiles.tile([128, 2, 512], bf16, tag="p")
                    if kc == 0:
                        # split the first exp so the first P@V starts half
                        # an activation earlier
                        nc.scalar.activation(
                            p2[:, :, lo:256], stp[:, :, lo:256], Exp
                        )
                        nc.scalar.activation(
                            p2[:, :, 256:], stp[:, :, 256:], Exp
                        )
                    else:
                        nc.scalar.activation(p2[:, :, lo:], stp[:, :, lo:], Exp)
                    if diag:
                        # zero the upper triangle of the 128-wide diagonal
                        # block (only this block straddles the mask)
                        ind2 = bass.AP(
                            tensor=indb.tensor, offset=indb.offset,
                            ap=[indb.ap[0], [0, 2], indb.ap[1]],
                        )
                        nc.vector.tensor_tensor(
                            out=p2[:, :, lo : lo + 128],
                            in0=p2[:, :, lo : lo + 128],
                            in1=ind2,
                            op=mult,
                        )
                    elif mode == "general":
                        base = m_sb[:, kc, :]
                        msk2 = bass.AP(
                            tensor=base.tensor,
                            offset=base.offset,
                            ap=[base.ap[0], [0, 2], base.ap[1]],
                        )
                        nc.vector.tensor_tensor(
                            out=p2, in0=p2, in1=msk2, op=mult
                        )
                    pieces = [(lo, 256), (256, 512)] if kc == 0 else [(lo, 512)]
                    for hh in range(2):
                        h = hp * 2 + hh
                        for pi, (c0, c1) in enumerate(pieces):
                            nc.tensor.matmul(
                                ops[hh][: D + 1, c0:c1],
                                lhsT=V1[:, kc, h, : D + 1],
                                rhs=p2[:, hh, c0:c1],
                                start=(kc == 0 and pi == 0),
                                stop=(kc == nkc - 1 and pi == len(pieces) - 1),
                            )
                    if state["emitted"] < nkc:
                        emit_mm1(state["emitted"])
                        state["emitted"] += 1

                def prologue():
                    state["emitted"] = min(2, nkc)  # lookahead 1
                    for j in range(state["emitted"]):
                        emit_mm1(j)

                def stash():
                    # OT casts (unnormalized) + denominator rows pulled to
                    # partition 0 by plain -64-shifted vector copies (no
                    # DMA: its ~4us SWDGE latency parked the reciprocal on
                    # the in-order vector queue and froze it)
                    ops = state["ops"]
                    qs = slice(qc * 512, (qc + 1) * 512)
                    sums = small.tile([1, 2, 512], f32, tag="sums", bufs=2)
                    nc.vector.tensor_copy(sums[0:1, 0, :], ops[0][D : D + 1, :])
                    nc.vector.tensor_copy(sums[0:1, 1, :], ops[1][D : D + 1, :])
                    if last:
                        # reciprocal jumps the vector queue ahead of the
                        # OT casts: it gates the whole drain chain
                        rcp = small.tile([1, 2, 512], f32, tag="rcpf", bufs=2)
                        nc.vector.reciprocal_approx_fast(out=rcp, in_=sums)
                        rcpb = small.tile([1, 2, 512], bf16, tag="rcpb", bufs=2)
                        nc.vector.tensor_copy(rcpb[0:1, 0, :], rcp[0:1, 0, :])
                        nc.scalar.copy(rcpb[0:1, 1, :], rcp[0:1, 1, :])
                        state["rcpb"] = rcpb
                        nc.scalar.copy(OT[0:64, hp, qs], ops[0][0:D, :])
                        nc.vector.tensor_copy(OT[64:128, hp, qs], ops[1][0:D, :])
                    else:
                        nc.vector.tensor_copy(OT[0:64, hp, qs], ops[0][0:D, :])
                        nc.vector.tensor_copy(OT[64:128, hp, qs], ops[1][0:D, :])
                    state["sums"] = sums

                def norm_a():
                    # reciprocal at partition 0 (broadcast + scale stay
                    # deferred in norm_b); the tail pair already did it
                    # inside stash
                    sums = state.pop("sums")
                    if last:
                        norm_state[(qc, hp)] = state.pop("rcpb")
                    else:
                        rcp = small.tile([1, 2, 512], f32, tag="rcpf", bufs=2)
                        nc.vector.reciprocal_approx_fast(out=rcp, in_=sums)
                        norm_state[(qc, hp)] = rcp

                units = [prologue]
                for kc in range(nkc):
                    units.append(lambda kc=kc: consume(kc))
                units.append(stash)
                units.append(norm_a)
                return units

            def weave_prologue(prev_units, next_units):
                """Move next pair's prologue before prev pair's last
                consume: its first S^T is ready (QT/KT resident, its stp
                frees with exp(nkc-2)) and fills the PE's wait on the
                last exp, instead of sitting behind PV(nkc-1) in the
                in-order PE queue."""
                prologue = next_units.pop(0)
                # prev_units = [prologue, c0..c(n-1), stash, norm_a]
                prev_units.insert(len(prev_units) - 3, prologue)

            def norm_b(qc, hp):
                # broadcast the partition-0 reciprocal rows (gpsimd) and
                # scale OT in place: hh=0 on vector, hh=1 on gpsimd (all
                # SBUF -- pool can't touch PSUM)
                rcp = norm_state.pop((qc, hp))
                qs = slice(qc * 512, (qc + 1) * 512)
                rb0 = small.tile([128, 512], f32, tag="rb0", bufs=2)
                nc.gpsimd.partition_broadcast(
                    rb0[0:64, :], rcp[0:1, 0, :], channels=64
                )
                rb1 = small.tile([128, 512], f32, tag="rb1", bufs=2)
                nc.gpsimd.partition_broadcast(
                    rb1, rcp[0:1, 1, :], channels=128
                )
                nc.vector.tensor_tensor(
                    out=OT[0:64, hp, qs], in0=OT[0:64, hp, qs],
                    in1=rb0[0:64, :], op=mult,
                )
                nc.vector.tensor_tensor(
                    out=OT[64:128, hp, qs], in0=OT[64:128, hp, qs],
                    in1=rb1[64:128, :], op=mult,
                )

            def norm_tail(qc, hp):
                # tail: broadcast via two one-row PE matmuls (contraction
                # 1, all operands at partition 0) so the gpsimd queue
                # never gates the drain; one full-width scale on vector
                rcpb = norm_state.pop((qc, hp))
                qs = slice(qc * 512, (qc + 1) * 512)
                bb = psum.tile(
                    [128, 2, 512], f32, name="ps1024", tag="ps1024", bufs=2
                )[:, 0, :]
                nc.tensor.matmul(
                    bb[0:64, :], lhsT=ones1, rhs=rcpb[0:1, 0, :],
                    start=True, stop=True,
                )
                nc.tensor.matmul(
                    bb[64:128, :], lhsT=ones1, rhs=rcpb[0:1, 1, :],
                    start=True, stop=True,
                )
                nc.vector.tensor_tensor(
                    out=OT[:, hp, qs], in0=OT[:, hp, qs], in1=bb, op=mult
                )

            # ---- schedule: staircase interleave ----
            # attn(qc) needs phase-1 chunks tb <= qc only, so phase-1(tb+1)
            # and proj(qc-1) units are injected between attention units to
            # keep the PE FIFO fed while ACT paces the exp chain.
            for u in p1_units(0, lambda kc: x0[:, kc, :]):
                u()
            nc.scalar.dma_start(
                out=Wp_sb, in_=Wp.rearrange("(dq p) n -> p dq n", p=128)
            )
            for qc in range(NQC):
                nkc = 4 * qc + 4 if mode == "causal" else NKC
                m_sb = None
                if mode == "general":
                    m_sb = xin.tile([128, NKC, 512], bf16, tag="mask", bufs=1)
                    nc.sync.dma_start(
                        out=m_sb,
                        in_=maskT.rearrange("(kc p) q -> p kc q", p=128)[
                            :, :, qc * 512 : (qc + 1) * 512
                        ],
                    )
                inj_early = []
                if qc + 1 < NTB:
                    x_next = xin.tile(
                        [128, 8, 512], bf16, tag="x_sb", name="x_sb"
                    )
                    def dma_u(tb=qc + 1, x_sb=x_next):
                        xr = xT.rearrange("(kc p) t -> p kc t", p=128)[
                            :, :, tb * 512 : (tb + 1) * 512
                        ]
                        for k2 in range(4):
                            nc.sync.dma_start(
                                out=x_sb[:, 2 * k2 : 2 * k2 + 2, :],
                                in_=xr[:, 2 * k2 : 2 * k2 + 2, :],
                            )
                    inj_early.append(dma_u)
                    inj_early += p1_units(
                        qc + 1, lambda kc, x_sb=x_next: x_sb[:, kc, :]
                    )
                # backload the output projection: qc3 is ACT-bound (32
                # full-width exps vs ~12us of attention matmuls), so spare
                # proj tiles are deferred there to keep the PE fed
                if qc == 1:
                    inj_late = proj_units(0)[0:2]
                elif qc == 2:
                    inj_late = proj_units(0)[2:4] + proj_units(1)[0:2]
                elif qc == 3:
                    inj_late = proj_units(1)[2:4] + proj_units(2)
                else:
                    inj_late = []
                hp_units = []
                for hp in range(NDQ):
                    hp_units.append(attn_units(qc, hp, nkc, m_sb))
                weave_prologue(hp_units[0], hp_units[1])
                # deferred norm: prior head-pair's broadcast+scale runs a
                # few units into the next block, when its reciprocal and
                # stash are done
                if qc > 0:
                    hp_units[0].insert(
                        3, lambda qc=qc: norm_b(qc - 1, 1)
                    )
                hp_units[1].insert(2, lambda qc=qc: norm_b(qc, 0))
                if qc == NQC - 1:
                    # pre-start the dq=0 half of three projection tiles:
                    # these matmuls fill the PE while the last pair's
                    # reciprocal chain runs off-engine
                    hp_units[1].append(lambda: proj_head(NTT - 4, big=True))
                    hp_units[1].append(lambda: proj_head(NTT - 3))
                    hp_units[1].append(lambda: proj_head(NTT - 2, use_ops=True))
                    hp_units[1].append(lambda qc=qc: norm_tail(qc, 1))
                main = hp_units[0] + hp_units[1]
                # the appended tail units (proj_head x3 + norm_tail) hold
                # all ps512 slots; no injections may land after them
                ntail = 4 if qc == NQC - 1 else 0
                half = (len(main) - ntail + 1) // 2
                mid = len(main) - ntail
                for part, inj in (
                    (main[:half], inj_early),
                    (main[half:mid], inj_late),
                ):
                    k, m, j = len(part), len(inj), 0
                    for i, u in enumerate(part):
                        u()
                        take = (i + 1) * m // k - i * m // k
                        for _ in range(take):
                            inj[j]()
                            j += 1
                for u in main[mid:]:
                    u()
            proj_finish(NTT - 4)
            proj_finish(NTT - 3)
            proj_finish(NTT - 2)
            for u in proj_units(NQC - 1, tail=True)[3:]:
                u()

            if debug_dump:
                nc.sync.dma_start(out=dbg["ot_d"], in_=OT)

    nc.compile()
    return nc


def _host_prep(x, prefix_causal_mask, W_attn, b_attn, W_proj):
    """Split full inputs into 8 per-core input maps; detect mask mode."""
    scale = 1.0 / np.sqrt(np.float32(D))
    mask = np.asarray(prefix_causal_mask)
    if mask.all():
        mode = "full"
    else:
        tri = np.tril(np.ones((T, T), dtype=bool))
        if all(np.array_equal(mask[b], tri) for b in range(B)):
            mode = "causal"
        else:
            mode = "general"

    import ml_dtypes

    bf16 = ml_dtypes.bfloat16
    x = np.asarray(x, dtype=np.float32)
    W_attn = np.asarray(W_attn, dtype=np.float32)
    b_attn = np.asarray(b_attn, dtype=np.float32)
    W_proj = np.asarray(W_proj, dtype=np.float32)

    in_maps = []
    for core in range(NCORES):
        b = core // NHG
        hg = core % NHG
        lo = hg * DL
        hi = lo + DL
        xT = np.ascontiguousarray(x[b].T)  # [C, T]
        Wq = W_attn[:, lo:hi] * scale
        Wk = W_attn[:, C + lo : C + hi]
        Wv = W_attn[:, 2 * C + lo : 2 * C + hi]
        Wl = np.concatenate([Wq, Wk], axis=1)  # [C, 512]
        # group-contiguous permutation: WgQK[p, g, kc, n] = Wl[kc*128+p, g*128+n]
        WgQK = np.ascontiguousarray(
            Wl.reshape(8, 128, 4, 128).transpose(1, 2, 0, 3)
        )
        WgV = np.ascontiguousarray(Wv.reshape(8, 128, 256).transpose(1, 0, 2))
        x0g = np.ascontiguousarray(
            xT[:, 0:512].reshape(8, 128, 512).transpose(1, 0, 2)
        )
        bq = b_attn[lo:hi] * scale
        bk = b_attn[C + lo : C + hi]
        # bias per partition for Q,K chunks: cols = [q0, q1, k0, k1]
        bqk = np.stack(
            [bq[0:128], bq[128:256], bk[0:128], bk[128:256]], axis=1
        ).astype(np.float32)
        bv = np.ascontiguousarray(
            b_attn[2 * C + lo : 2 * C + hi][None, :]
        ).astype(np.float32)
        Wp = np.ascontiguousarray(W_proj[lo:hi, :])
        im = {
            "xT": xT.astype(bf16),
            "WgQK": WgQK.astype(bf16),
            "WgV": WgV.astype(bf16),
            "x0g": x0g.astype(bf16),
            "bqk": np.ascontiguousarray(bqk),
            "bv": bv,
            "Wp": Wp.astype(bf16),
        }
        if mode == "general":
            im["maskT"] = np.ascontiguousarray(mask[b].T).astype(bf16)
        in_maps.append(im)
    return mode, in_maps


def _get_program(mode):
    if mode not in _CACHE:
        _CACHE[mode] = _build(mode)
    return _CACHE[mode]


def _run(inputs, trace=False):
    """Returns (full_output [B,T,C], BassKernelResults)."""
    from concourse import bass_utils

    mode, in_maps = _host_prep(
        inputs["x"],
        inputs["prefix_causal_mask"],
        inputs["W_attn"],
        inputs["b_attn"],
        inputs["W_proj"],
    )
    nc = _get_program(mode)
    res = bass_utils.run_bass_kernel_spmd(
        nc, in_maps, core_ids=list(range(NCORES)), trace=trace
    )
    b_proj = np.asarray(inputs["b_proj"], dtype=np.float32)
    y = np.zeros((B, T, C), dtype=np.float32)
    for core in range(NCORES):
        y[core // NHG] += np.asarray(res.results[core]["yp"], dtype=np.float32)
    y += b_proj[None, None, :]
    return y, res


def kernel(**inputs):
    y, _ = _run(inputs, trace=False)
    return y


# revision 65
# speedup vs baseline: 1.0220x; 1.0220x over previous
"""Causal self-attention for Trainium2, 8 NeuronCores.

Sharding: tensor-parallel over heads (4 heads/core) x data-parallel over
batch (2). Core i handles batch i//4, heads 4*(i%4)..4*(i%4)+3. Each core
computes its heads' attention output and a partial output projection
(W_proj rows for its heads); the host sums the 4 partials per batch and
adds b_proj.

Device layout choices:
  - Q^T, K^T computed feature-major [dim, t] directly (lhsT = W chunk,
    rhs = x^T chunk), so attention scores come out as S^T [k, q] with k
    on partitions -- which is exactly the layout the P@V matmul needs
    as its rhs. No on-chip transposes of the O(T^2) object.
  - V computed in natural [t, dim] layout (lhsT = x^T chunk, rhs = W_v),
    which is the lhsT layout the P@V matmul needs. A ones-column is
    appended to V so the softmax denominators fall out of the same
    matmul (PSUM partition 64 of each head's accumulator).
  - exp() without max subtraction: scores are q.k/8 with q,k ~ N(0,1),
    bounded well inside fp32 exp range; softmax is shift-invariant so
    the result is mathematically identical to the reference.

Causal handling: fully-masked k-chunks are skipped; on diagonal chunks
the S^T and P@V matmuls are column-trimmed to the unmasked q-range
(PSUM zero-regions are bank-row sized, so a full-width start followed
by narrower accumulates is legal), and the 128-wide triangular block
is zeroed after the exp by one shared 0/1 indicator tile. If the
runtime mask is not the lower-tri causal mask, a general fallback
multiplies p by the actual mask (DMA'd transposed) after the exp; an
all-ones mask drops masking entirely.

Engine budget (the PE is the bottleneck at ~124us busy; everything else
is placed to never stall it): PE does matmuls only; ACT does the exp
chain (~75us, the co-bottleneck in the last q-block) plus a couple of
tail copies; DVE does all PSUM-reading elementwise work (QK/V bias
casts, stash casts, projection casts, diag masks, reciprocals) -- the
Pool/GpSimd engine does only reciprocal partition-broadcasts and DMA
dispatch, because on TRN2 it cannot touch PSUM and its tensor ops run
on slow Q7 DSP cores.

Softmax denominators: V's ones-column puts sum(p) on PSUM partition 64
of each head's P@V accumulator; plain -64-partition-shifted vector
copies pull the two rows to partition 0 (custom-DVE reciprocal and
partition_broadcast silently mis-read non-zero partition bases on HW,
and a gpsimd DMA here costs ~4us of SWDGE latency that freezes the
in-order vector queue), then reciprocal + gpsimd broadcast + in-place
OT scale, all deferred a few units so nothing waits. The tail pair
instead broadcasts via two one-row PE matmuls (contraction 1, all
operands at partition 0) so the gpsimd queue never gates the drain,
while pre-started dq0 projection matmuls keep the PE's p-state hot
(TRN2's PE clock ramps 0.65->1.2->2.4GHz with ~3us of continuous
execution and resets on every idle gap -- sustained feed beats an
early trickle, which is also why the startup DMA is batched in pairs).

Scheduling: the output projection is backloaded (2/4/6 tiles injected
into blocks qc1/qc2/qc3) because qc3's exp work (~27us) far exceeds
its attention matmul work (~12us) -- the spare projection tiles keep
the PE busy while ACT grinds through the last exps.
"""

import numpy as np

B, T, C, H = 2, 2048, 1024, 16
D = C // H            # 64 head dim
NCORES = 8
NBG = 2               # batch shards
NHG = 4               # head-group shards
HL = H // NHG         # 4 heads per core
DL = HL * D           # 256 local feature dims
NDQ = DL // 128       # 2 partition chunks of local dims
NTB = T // 512        # 4 t-chunks of 512
NKC = T // 128        # 16 key chunks of 128
NQC = T // 512        # 4 query chunks of 512
NTT = T // 128        # 16 t-tiles of 128 (proj / V)

_CACHE = {}


def _build(mode, debug_dump=False):
    """Build + compile the per-core Bass program. mode: causal|full|general."""
    import concourse.bass as bass
    import concourse.bacc as bacc
    import concourse.tile as tile
    import concourse.mybir as mybir

    f32 = mybir.dt.float32
    bf16 = mybir.dt.bfloat16
    Exp = mybir.ActivationFunctionType.Exp
    mult = mybir.AluOpType.mult
    add = mybir.AluOpType.add

    nc = bacc.Bacc(
        "TRN2", target_bir_lowering=False, debug=False, num_devices=NCORES
    )

    xT = nc.dram_tensor("xT", [C, T], bf16, kind="ExternalInput").ap()
    # W pre-permuted on the host so each column group is contiguous per
    # partition (2KB+ DMA lines; the naive [C, 3*DL] layout gives 256B
    # strided segments and ~5us group loads)
    WgQK = nc.dram_tensor(
        "WgQK", [128, 4, 8, 128], bf16, kind="ExternalInput"
    ).ap()
    WgV = nc.dram_tensor("WgV", [128, 8, 256], bf16, kind="ExternalInput").ap()
    # x's first t-chunk, host-permuted contiguous per partition (8KB DMA
    # lines): the startup is DMA-bound, later t-chunks stream fine from xT
    x0g = nc.dram_tensor("x0g", [128, 8, 512], bf16, kind="ExternalInput").ap()
    bqk = nc.dram_tensor("bqk", [128, 2 * NDQ], f32, kind="ExternalInput").ap()
    bv = nc.dram_tensor("bv", [1, DL], f32, kind="ExternalInput").ap()
    Wp = nc.dram_tensor("Wp", [DL, C], bf16, kind="ExternalInput").ap()
    maskT = None
    if mode == "general":
        maskT = nc.dram_tensor("maskT", [T, T], bf16, kind="ExternalInput").ap()
    yp = nc.dram_tensor("yp", [T, C], bf16, kind="ExternalOutput").ap()
    dbg = {}
    if debug_dump:
        for nm, shp, dt in [
            ("ot_d", [128, NDQ, T], bf16),
        ]:
            dbg[nm] = nc.dram_tensor(nm, shp, dt, kind="ExternalOutput").ap()

    with tile.TileContext(nc) as tc:
        with (
            tc.tile_pool(name="singles", bufs=1) as singles,
            tc.tile_pool(name="xin", bufs=2) as xin,
            tc.tile_pool(name="ptiles", bufs=8) as ptiles,
            tc.tile_pool(name="small", bufs=4) as small,
            tc.tile_pool(name="outp", bufs=4) as outp,
            tc.tile_pool(name="psum", bufs=7, space="PSUM") as psum,
        ):
            def ps512(name):
                # transient accumulators (qk/v/proj): 2-bank rotation
                return psum.tile(
                    [128, 512], f32, name="ps512", tag="ps512", bufs=2
                )

            def ps_ops(name):
                # attention P@V accumulators live in their own 2-bank tag:
                # they are held for a whole head-pair, and sharing a
                # rotation with the transients chained every third
                # injected unit onto a pair-long hold
                return psum.tile(
                    [128, 512], f32, name="ops", tag="ops", bufs=2
                )

            # ---- resident inputs ----
            # W loads by COLUMN GROUP in unit-consumption order (Q0, Q1,
            # K0, K1, V) on the sync queue; x's first t-chunk rides the
            # scalar (ACT) hwdge queue -- idle until the first exp -- so
            # the two dispatch chains run in parallel and the first QK
            # unit's operands land ~2us after the preamble.
            x0 = xin.tile([128, 8, 512], bf16, tag="x0", bufs=1)
            # three parallel dispatch chains: W column groups + first x
            # pairs on sync (both host-permuted contiguous per partition),
            # the rest of x then the small inputs on scalar
            Wq0 = singles.tile([128, 8, 128], bf16, name="Wq0")
            nc.sync.dma_start(out=x0[:, 0:2, :], in_=x0g[:, 0:2, :])
            nc.sync.dma_start(out=Wq0[:, 0:3, :], in_=WgQK[:, 0, 0:3, :])
            nc.sync.dma_start(out=x0[:, 2:4, :], in_=x0g[:, 2:4, :])
            nc.sync.dma_start(out=Wq0[:, 3:8, :], in_=WgQK[:, 0, 3:8, :])
            nc.scalar.dma_start(out=x0[:, 4:6, :], in_=x0g[:, 4:6, :])
            nc.scalar.dma_start(out=x0[:, 6:8, :], in_=x0g[:, 6:8, :])
            bqk_sb = singles.tile([128, 2 * NDQ], f32)
            nc.scalar.dma_start(out=bqk_sb, in_=bqk)
            bv_row = singles.tile([1, DL], f32)
            nc.scalar.dma_start(out=bv_row, in_=bv)
            Wcg = [Wq0]
            for gi in range(1, 4):
                wt = singles.tile([128, 8, 128], bf16, name=f"Wcg{gi}")
                nc.sync.dma_start(out=wt, in_=WgQK[:, gi, :, :])
                Wcg.append(wt)
            WV = singles.tile([128, 8, 256], bf16, name="WV")
            nc.sync.dma_start(out=WV, in_=WgV)

            bv_sb = singles.tile([128, DL], f32)
            nc.gpsimd.partition_broadcast(bv_sb, bv_row)

            # 0/1 indicator for the 128-wide diagonal block: every diag
            # chunk uses the same within-block mask (keep iff c >= p)
            indb = None
            if mode == "causal":
                indb = singles.tile([128, 128], bf16, name="indb")
                nc.vector.memset(indb, 1.0)
                nc.gpsimd.affine_select(
                    out=indb, in_=indb,
                    compare_op=mybir.AluOpType.is_ge,
                    fill=0.0, base=0, pattern=[[1, 128]],
                    channel_multiplier=-1,
                )

            # ones row for the tail's one-row broadcast matmuls
            ones1 = singles.tile([1, 64], bf16, name="ones1")
            nc.vector.memset(ones1, 1.0)

            # ---- resident intermediates ----
            QT = singles.tile([128, NDQ, T], bf16)   # [dim%128, dimchunk, t]
            KT = singles.tile([128, NDQ, T], bf16)
            # V plus a ones-column: each head uses cols [0:D+1], so its
            # softmax denominator lands on PSUM partition 64.
            V1 = singles.tile([128, NKC, HL, D + 1], bf16)
            nc.vector.memset(V1[:, :, :, D : D + 1], 1.0)
            OT = singles.tile([128, NDQ, T], bf16)
            Wp_sb = singles.tile([128, NDQ, C], bf16)

            # ---- phase 1: QKV projections (as interleavable units) ----
            def p1_units(tb, xs):
                """Units for one 512-wide t-chunk of the QKV projection.
                xs(kc) -> [128, 512] AP for contraction chunk kc."""
                units = []
                for s in range(2):  # 0=Q, 1=K
                    for dq in range(NDQ):
                        def qk_u(tb=tb, s=s, dq=dq, xs=xs):
                            ps = ps512("qk")
                            g = s * NDQ + dq
                            for kc in range(8):
                                nc.tensor.matmul(
                                    ps,
                                    lhsT=Wcg[g][:, kc, :],
                                    rhs=xs(kc),
                                    start=(kc == 0),
                                    stop=(kc == 7),
                                )
                            dst = (QT if s == 0 else KT)[
                                :, dq, tb * 512 : (tb + 1) * 512
                            ]
                            nc.vector.tensor_scalar_add(
                                dst, ps,
                                bqk_sb[:, g : g + 1],
                            )
                        units.append(qk_u)
                for t4 in range(4):
                    def v_u(tb=tb, t4=t4, xs=xs):
                        tt = tb * 4 + t4
                        ps = ps512("v")
                        for kc in range(8):
                            nc.tensor.matmul(
                                ps[:, :DL],
                                lhsT=xs(kc)[:, t4 * 128 : (t4 + 1) * 128],
                                rhs=WV[:, kc, :],
                                start=(kc == 0),
                                stop=(kc == 7),
                            )
                        nc.vector.tensor_tensor(
                            out=V1[:, tt, :, 0:D],
                            in0=ps[:, :DL].rearrange("p (h d) -> p h d", d=D),
                            in1=bv_sb.rearrange("p (h d) -> p h d", d=D),
                            op=add,
                        )
                    units.append(v_u)
                return units

            # ---- projection units ----
            def proj_units(qc, tail=False, use_scalar=False):
                units = []
                for t4 in range(4):
                    def u(qc=qc, t4=t4, tail=tail):
                        tt = qc * 4 + t4
                        y_sb = outp.tile([128, C], bf16, name="y_sb")
                        for n in range(2):
                            pp = ps512("proj")
                            for dq in range(NDQ):
                                nc.tensor.matmul(
                                    pp,
                                    lhsT=OT[:, dq, tt * 128 : (tt + 1) * 128],
                                    rhs=Wp_sb[:, dq, n * 512 : (n + 1) * 512],
                                    start=(dq == 0),
                                    stop=(dq == NDQ - 1),
                                )
                            dst = y_sb[:, n * 512 : (n + 1) * 512]
                            if (tail or use_scalar) and n == 1:
                                nc.scalar.copy(dst, pp)
                            else:
                                nc.vector.tensor_copy(dst, pp)
                        if tail:
                            for n in range(2):
                                nc.sync.dma_start(
                                    out=yp[tt * 128 : (tt + 1) * 128,
                                           n * 512 : (n + 1) * 512],
                                    in_=y_sb[:, n * 512 : (n + 1) * 512],
                                )
                        else:
                            nc.sync.dma_start(
                                out=yp[tt * 128 : (tt + 1) * 128, :], in_=y_sb
                            )
                    units.append(u)
                return units

            # tail-only split projection: dq=0 accumulation pre-started
            # while the last pair's reciprocal/broadcast chain runs
            pp_state = {}

            def proj_head(tt, big=False, use_ops=False):
                if big:
                    tb = psum.tile(
                        [128, 2, 512], f32, name="ps1024", tag="ps1024",
                        bufs=2,
                    )
                for n in range(2):
                    pp = tb[:, n, :] if big else (
                        ps_ops("projA") if use_ops else ps512("projA")
                    )
                    nc.tensor.matmul(
                        pp,
                        lhsT=OT[:, 0, tt * 128 : (tt + 1) * 128],
                        rhs=Wp_sb[:, 0, n * 512 : (n + 1) * 512],
                        start=True,
                        stop=False,
                    )
                    pp_state[(tt, n)] = pp

            def proj_finish(tt):
                y_sb = outp.tile([128, C], bf16, name="y_sb")
                for n in range(2):
                    pp = pp_state.pop((tt, n))
                    nc.tensor.matmul(
                        pp,
                        lhsT=OT[:, 1, tt * 128 : (tt + 1) * 128],
                        rhs=Wp_sb[:, 1, n * 512 : (n + 1) * 512],
                        start=False,
                        stop=True,
                    )
                    dst = y_sb[:, n * 512 : (n + 1) * 512]
                    if n == 1:
                        nc.scalar.copy(dst, pp)
                    else:
                        nc.vector.tensor_copy(dst, pp)
                for n in range(2):
                    nc.sync.dma_start(
                        out=yp[tt * 128 : (tt + 1) * 128,
                               n * 512 : (n + 1) * 512],
                        in_=y_sb[:, n * 512 : (n + 1) * 512],
                    )

            norm_state = {}

            def attn_units(qc, hp, nkc, m_sb):
                """One head-pair's attention over all k-chunks; softmax
                denominators ride V's ones-column onto PSUM partition 64."""
                state = {}
                last = qc == NQC - 1 and hp == NDQ - 1

                def lo_of(kc):
                    if mode == "causal" and kc >= 4 * qc:
                        return 128 * (kc - 4 * qc)
                    return 0

                def emit_mm1(j):
                    lo = lo_of(j)
                    stp = psum.tile(
                        [128, 2, 512], f32, name="ps1024", tag="ps1024", bufs=2
                    )
                    for hh in range(2):
                        off = 64 * hh
                        nc.tensor.matmul(
                            stp[:, hh, lo:],
                            lhsT=KT[off : off + 64, hp, j * 128 : (j + 1) * 128],
                            rhs=QT[
                                off : off + 64, hp,
                                qc * 512 + lo : (qc + 1) * 512,
                            ],
                            start=True,
                            stop=True,
                        )
                    state.setdefault("st", {})[j] = stp

                def consume(kc):
                    if kc == 0:
                        state["ops"] = [ps_ops("o"), ps_ops("o")]
                    ops = state["ops"]
                    stp = state["st"].pop(kc)
                    lo = lo_of(kc)
                    diag = mode == "causal" and kc >= 4 * qc
                    p2 = ptiles.tile([128, 2, 512], bf16, tag="p")
                    if kc == 0:
                        # split the first exp so the first P@V starts half
                        # an activation earlier
                        nc.scalar.activation(
                            p2[:, :, lo:256], stp[:, :, lo:256], Exp
                        )
                        nc.scalar.activation(
                            p2[:, :, 256:], stp[:, :, 256:], Exp
                        )
                    else:
                        nc.scalar.activation(p2[:, :, lo:], stp[:, :, lo:], Exp)
                    if diag:
                        # zero the upper triangle of the 128-wide diagonal
                        # block (only this block straddles the mask)
                        ind2 = bass.AP(
                            tensor=indb.tensor, offset=indb.offset,
                            ap=[indb.ap[0], [0, 2], indb.ap[1]],
                        )
                        nc.vector.tensor_tensor(
                            out=p2[:, :, lo : lo + 128],
                            in0=p2[:, :, lo : lo + 128],
                            in1=ind2,
                            op=mult,
                        )
                    elif mode == "general":
                        base = m_sb[:, kc, :]
                        msk2 = bass.AP(
                            tensor=base.tensor,
                            offset=base.offset,
                            ap=[base.ap[0], [0, 2], base.ap[1]],
                        )
                        nc.vector.tensor_tensor(
                            out=p2, in0=p2, in1=msk2, op=mult
                        )
                    pieces = [(lo, 256), (256, 512)] if kc == 0 else [(lo, 512)]
                    for hh in range(2):
                        h = hp * 2 + hh
                        for pi, (c0, c1) in enumerate(pieces):
                            nc.tensor.matmul(
                                ops[hh][: D + 1, c0:c1],
                                lhsT=V1[:, kc, h, : D + 1],
                                rhs=p2[:, hh, c0:c1],
                                start=(kc == 0 and pi == 0),
                                stop=(kc == nkc - 1 and pi == len(pieces) - 1),
                            )
                    if state["emitted"] < nkc:
                        emit_mm1(state["emitted"])
                        state["emitted"] += 1

                def prologue(only=None):
                    if only is not None:
                        emit_mm1(only)
                        state["emitted"] = only + 1
                        return
                    state["emitted"] = min(2, nkc)  # lookahead 1
                    for j in range(state["emitted"]):
                        emit_mm1(j)

                def stash():
                    # OT casts (unnormalized) + denominator rows pulled to
                    # partition 0 by plain -64-shifted vector copies (no
                    # DMA: its ~4us SWDGE latency parked the reciprocal on
                    # the in-order vector queue and froze it)
                    ops = state["ops"]
                    qs = slice(qc * 512, (qc + 1) * 512)
                    sums = small.tile([1, 2, 512], f32, tag="sums", bufs=2)
                    nc.vector.tensor_copy(sums[0:1, 0, :], ops[0][D : D + 1, :])
                    nc.vector.tensor_copy(sums[0:1, 1, :], ops[1][D : D + 1, :])
                    if last:
                        # reciprocal jumps the vector queue ahead of the
                        # OT casts: it gates the whole drain chain
                        rcp = small.tile([1, 2, 512], f32, tag="rcpf", bufs=2)
                        nc.vector.reciprocal_approx_fast(out=rcp, in_=sums)
                        rcpb = small.tile([1, 2, 512], bf16, tag="rcpb", bufs=2)
                        nc.vector.tensor_copy(rcpb[0:1, 0, :], rcp[0:1, 0, :])
                        nc.scalar.copy(rcpb[0:1, 1, :], rcp[0:1, 1, :])
                        state["rcpb"] = rcpb
                        nc.scalar.copy(OT[0:64, hp, qs], ops[0][0:D, :])
                        nc.vector.tensor_copy(OT[64:128, hp, qs], ops[1][0:D, :])
                    else:
                        nc.vector.tensor_copy(OT[0:64, hp, qs], ops[0][0:D, :])
                        nc.vector.tensor_copy(OT[64:128, hp, qs], ops[1][0:D, :])
                    state["sums"] = sums

                def norm_a():
                    # reciprocal at partition 0 (broadcast + scale stay
                    # deferred in norm_b); the tail pair already did it
                    # inside stash
                    sums = state.pop("sums")
                    if last:
                        norm_state[(qc, hp)] = state.pop("rcpb")
                    else:
                        rcp = small.tile([1, 2, 512], f32, tag="rcpf", bufs=2)
                        nc.vector.reciprocal_approx_fast(out=rcp, in_=sums)
                        norm_state[(qc, hp)] = rcp

                units = [prologue]
                for kc in range(nkc):
                    units.append(lambda kc=kc: consume(kc))
                units.append(stash)
                units.append(norm_a)
                return units

            def weave_prologue(prev_units, next_units):
                """Split next pair's prologue and move only its FIRST
                S^T before prev pair's last consume: it is ready (QT/KT
                resident, its stp frees with exp(nkc-2)) and fills the
                PE's wait on the last exp instead of sitting behind
                PV(nkc-1) in the in-order PE queue. The second S^T stays
                at pair start (it waits the same event as PV(nkc-1) and
                would delay it)."""
                prologue = next_units.pop(0)

                def prologue_a():
                    prologue(only=0)

                def prologue_b():
                    prologue(only=1)

                next_units.insert(0, prologue_b)
                # prev_units = [prologue, c0..c(n-1), stash, norm_a]
                prev_units.insert(len(prev_units) - 3, prologue_a)

            def norm_b(qc, hp):
                # broadcast the partition-0 reciprocal rows (gpsimd) and
                # scale OT in place: hh=0 on vector, hh=1 on gpsimd (all
                # SBUF -- pool can't touch PSUM)
                rcp = norm_state.pop((qc, hp))
                qs = slice(qc * 512, (qc + 1) * 512)
                rb0 = small.tile([128, 512], f32, tag="rb0", bufs=2)
                nc.gpsimd.partition_broadcast(
                    rb0[0:64, :], rcp[0:1, 0, :], channels=64
                )
                rb1 = small.tile([128, 512], f32, tag="rb1", bufs=2)
                nc.gpsimd.partition_broadcast(
                    rb1, rcp[0:1, 1, :], channels=128
                )
                nc.vector.tensor_tensor(
                    out=OT[0:64, hp, qs], in0=OT[0:64, hp, qs],
                    in1=rb0[0:64, :], op=mult,
                )
                nc.vector.tensor_tensor(
                    out=OT[64:128, hp, qs], in0=OT[64:128, hp, qs],
                    in1=rb1[64:128, :], op=mult,
                )

            def norm_tail(qc, hp):
                # tail: broadcast via two one-row PE matmuls (contraction
                # 1, all operands at partition 0) so the gpsimd queue
                # never gates the drain; one full-width scale on vector
                rcpb = norm_state.pop((qc, hp))
                qs = slice(qc * 512, (qc + 1) * 512)
                bb = psum.tile(
                    [128, 2, 512], f32, name="ps1024", tag="ps1024", bufs=2
                )[:, 0, :]
                nc.tensor.matmul(
                    bb[0:64, :], lhsT=ones1, rhs=rcpb[0:1, 0, :],
                    start=True, stop=True,
                )
                nc.tensor.matmul(
                    bb[64:128, :], lhsT=ones1, rhs=rcpb[0:1, 1, :],
                    start=True, stop=True,
                )
                nc.vector.tensor_tensor(
                    out=OT[:, hp, qs], in0=OT[:, hp, qs], in1=bb, op=mult
                )

            # ---- schedule: staircase interleave ----
            # attn(qc) needs phase-1 chunks tb <= qc only, so phase-1(tb+1)
            # and proj(qc-1) units are injected between attention units to
            # keep the PE FIFO fed while ACT paces the exp chain.
            for u in p1_units(0, lambda kc: x0[:, kc, :]):
                u()
            nc.scalar.dma_start(
                out=Wp_sb, in_=Wp.rearrange("(dq p) n -> p dq n", p=128)
            )
            for qc in range(NQC):
                nkc = 4 * qc + 4 if mode == "causal" else NKC
                m_sb = None
                if mode == "general":
                    m_sb = xin.tile([128, NKC, 512], bf16, tag="mask", bufs=1)
                    nc.sync.dma_start(
                        out=m_sb,
                        in_=maskT.rearrange("(kc p) q -> p kc q", p=128)[
                            :, :, qc * 512 : (qc + 1) * 512
                        ],
                    )
                inj_early = []
                if qc + 1 < NTB:
                    x_next = xin.tile(
                        [128, 8, 512], bf16, tag="x_sb", name="x_sb"
                    )
                    def dma_u(tb=qc + 1, x_sb=x_next):
                        xr = xT.rearrange("(kc p) t -> p kc t", p=128)[
                            :, :, tb * 512 : (tb + 1) * 512
                        ]
                        for k2 in range(4):
                            nc.sync.dma_start(
                                out=x_sb[:, 2 * k2 : 2 * k2 + 2, :],
                                in_=xr[:, 2 * k2 : 2 * k2 + 2, :],
                            )
                    inj_early.append(dma_u)
                    inj_early += p1_units(
                        qc + 1, lambda kc, x_sb=x_next: x_sb[:, kc, :]
                    )
                # backload the output projection: qc3 is ACT-bound (32
                # full-width exps vs ~12us of attention matmuls), so spare
                # proj tiles are deferred there to keep the PE fed
                if qc == 1:
                    inj_late = proj_units(0)[0:2]
                elif qc == 2:
                    inj_late = proj_units(0)[2:4] + proj_units(1)[0:2]
                elif qc == 3:
                    inj_late = proj_units(1)[2:4] + proj_units(2)
                else:
                    inj_late = []
                hp_units = []
                for hp in range(NDQ):
                    hp_units.append(attn_units(qc, hp, nkc, m_sb))
                weave_prologue(hp_units[0], hp_units[1])
                # deferred norm: prior head-pair's broadcast+scale runs a
                # few units into the next block, when its reciprocal and
                # stash are done
                if qc > 0:
                    hp_units[0].insert(
                        3, lambda qc=qc: norm_b(qc - 1, 1)
                    )
                hp_units[1].insert(3, lambda qc=qc: norm_b(qc, 0))
                if qc == NQC - 1:
                    # pre-start the dq=0 half of three projection tiles:
                    # these matmuls fill the PE while the last pair's
                    # reciprocal chain runs off-engine
                    hp_units[1].append(lambda: proj_head(NTT - 4, big=True))
                    hp_units[1].append(lambda: proj_head(NTT - 3))
                    hp_units[1].append(lambda: proj_head(NTT - 2, use_ops=True))
                    hp_units[1].append(lambda qc=qc: norm_tail(qc, 1))
                main = hp_units[0] + hp_units[1]
                # the appended tail units (proj_head x3 + norm_tail) hold
                # all ps512 slots; no injections may land after them
                ntail = 4 if qc == NQC - 1 else 0
                half = (len(main) - ntail + 1) // 2
                mid = len(main) - ntail
                for part, inj in (
                    (main[:half], inj_early),
                    (main[half:mid], inj_late),
                ):
                    k, m, j = len(part), len(inj), 0
                    for i, u in enumerate(part):
                        u()
                        take = (i + 1) * m // k - i * m // k
                        for _ in range(take):
                            inj[j]()
                            j += 1
                for u in main[mid:]:
                    u()
            proj_finish(NTT - 4)
            proj_finish(NTT - 3)
            proj_finish(NTT - 2)
            for u in proj_units(NQC - 1, tail=True)[3:]:
                u()

            if debug_dump:
                nc.sync.dma_start(out=dbg["ot_d"], in_=OT)

    nc.compile()
    return nc


def _host_prep(x, prefix_causal_mask, W_attn, b_attn, W_proj):
    """Split full inputs into 8 per-core input maps; detect mask mode."""
    scale = 1.0 / np.sqrt(np.float32(D))
    mask = np.asarray(prefix_causal_mask)
    if mask.all():
        mode = "full"
    else:
        tri = np.tril(np.ones((T, T), dtype=bool))
        if all(np.array_equal(mask[b], tri) for b in range(B)):
            mode = "causal"
        else:
            mode = "general"

    import ml_dtypes

    bf16 = ml_dtypes.bfloat16
    x = np.asarray(x, dtype=np.float32)
    W_attn = np.asarray(W_attn, dtype=np.float32)
    b_attn = np.asarray(b_attn, dtype=np.float32)
    W_proj = np.asarray(W_proj, dtype=np.float32)

    in_maps = []
    for core in range(NCORES):
        b = core // NHG
        hg = core % NHG
        lo = hg * DL
        hi = lo + DL
        xT = np.ascontiguousarray(x[b].T)  # [C, T]
        Wq = W_attn[:, lo:hi] * scale
        Wk = W_attn[:, C + lo : C + hi]
        Wv = W_attn[:, 2 * C + lo : 2 * C + hi]
        Wl = np.concatenate([Wq, Wk], axis=1)  # [C, 512]
        # group-contiguous permutation: WgQK[p, g, kc, n] = Wl[kc*128+p, g*128+n]
        WgQK = np.ascontiguousarray(
            Wl.reshape(8, 128, 4, 128).transpose(1, 2, 0, 3)
        )
        WgV = np.ascontiguousarray(Wv.reshape(8, 128, 256).transpose(1, 0, 2))
        x0g = np.ascontiguousarray(
            xT[:, 0:512].reshape(8, 128, 512).transpose(1, 0, 2)
        )
        bq = b_attn[lo:hi] * scale
        bk = b_attn[C + lo : C + hi]
        # bias per partition for Q,K chunks: cols = [q0, q1, k0, k1]
        bqk = np.stack(
            [bq[0:128], bq[128:256], bk[0:128], bk[128:256]], axis=1
        ).astype(np.float32)
        bv = np.ascontiguousarray(
            b_attn[2 * C + lo : 2 * C + hi][None, :]
        ).astype(np.float32)
        Wp = np.ascontiguousarray(W_proj[lo:hi, :])
        im = {
            "xT": xT.astype(bf16),
            "WgQK": WgQK.astype(bf16),
            "WgV": WgV.astype(bf16),
            "x0g": x0g.astype(bf16),
            "bqk": np.ascontiguousarray(bqk),
            "bv": bv,
            "Wp": Wp.astype(bf16),
        }
        if mode == "general":
            im["maskT"] = np.ascontiguousarray(mask[b].T).astype(bf16)
        in_maps.append(im)
    return mode, in_maps


def _get_program(mode):
    if mode not in _CACHE:
        _CACHE[mode] = _build(mode)
    return _CACHE[mode]


def _run(inputs, trace=False):
    """Returns (full_output [B,T,C], BassKernelResults)."""
    from concourse import bass_utils

    mode, in_maps = _host_prep(
        inputs["x"],
        inputs["prefix_causal_mask"],
        inputs["W_attn"],
        inputs["b_attn"],
        inputs["W_proj"],
    )
    nc = _get_program(mode)
    res = bass_utils.run_bass_kernel_spmd(
        nc, in_maps, core_ids=list(range(NCORES)), trace=trace
    )
    b_proj = np.asarray(inputs["b_proj"], dtype=np.float32)
    y = np.zeros((B, T, C), dtype=np.float32)
    for core in range(NCORES):
        y[core // NHG] += np.asarray(res.results[core]["yp"], dtype=np.float32)
    y += b_proj[None, None, :]
    return y, res


def kernel(**inputs):
    y, _ = _run(inputs, trace=False)
    return y


# revision 67
# speedup vs baseline: 1.0312x; 1.0090x over previous
"""Causal self-attention for Trainium2, 8 NeuronCores.

Sharding: tensor-parallel over heads (4 heads/core) x data-parallel over
batch (2). Core i handles batch i//4, heads 4*(i%4)..4*(i%4)+3. Each core
computes its heads' attention output and a partial output projection
(W_proj rows for its heads); the host sums the 4 partials per batch and
adds b_proj.

Device layout choices:
  - Q^T, K^T computed feature-major [dim, t] directly (lhsT = W chunk,
    rhs = x^T chunk), so attention scores come out as S^T [k, q] with k
    on partitions -- which is exactly the layout the P@V matmul needs
    as its rhs. No on-chip transposes of the O(T^2) object.
  - V computed in natural [t, dim] layout (lhsT = x^T chunk, rhs = W_v),
    which is the lhsT layout the P@V matmul needs. A ones-column is
    appended to V so the softmax denominators fall out of the same
    matmul (PSUM partition 64 of each head's accumulator).
  - exp() without max subtraction: scores are q.k/8 with q,k ~ N(0,1),
    bounded well inside fp32 exp range; softmax is shift-invariant so
    the result is mathematically identical to the reference.

Causal handling: fully-masked k-chunks are skipped; on diagonal chunks
the S^T and P@V matmuls are column-trimmed to the unmasked q-range
(PSUM zero-regions are bank-row sized, so a full-width start followed
by narrower accumulates is legal), and the 128-wide triangular block
is zeroed after the exp by one shared 0/1 indicator tile. If the
runtime mask is not the lower-tri causal mask, a general fallback
multiplies p by the actual mask (DMA'd transposed) after the exp; an
all-ones mask drops masking entirely.

Engine budget (the PE is the bottleneck at ~124us busy; everything else
is placed to never stall it): PE does matmuls only; ACT does the exp
chain (~75us, the co-bottleneck in the last q-block) plus a couple of
tail copies; DVE does all PSUM-reading elementwise work (QK/V bias
casts, stash casts, projection casts, diag masks, reciprocals) -- the
Pool/GpSimd engine does only reciprocal partition-broadcasts and DMA
dispatch, because on TRN2 it cannot touch PSUM and its tensor ops run
on slow Q7 DSP cores.

Softmax denominators: V's ones-column puts sum(p) on PSUM partition 64
of each head's P@V accumulator; plain -64-partition-shifted vector
copies pull the two rows to partition 0 (custom-DVE reciprocal and
partition_broadcast silently mis-read non-zero partition bases on HW,
and a gpsimd DMA here costs ~4us of SWDGE latency that freezes the
in-order vector queue), then reciprocal + gpsimd broadcast + in-place
OT scale, all deferred a few units so nothing waits. The tail pair
instead broadcasts via two one-row PE matmuls (contraction 1, all
operands at partition 0) so the gpsimd queue never gates the drain,
while pre-started dq0 projection matmuls keep the PE's p-state hot
(TRN2's PE clock ramps 0.65->1.2->2.4GHz with ~3us of continuous
execution and resets on every idle gap -- sustained feed beats an
early trickle, which is also why the startup DMA is batched in pairs).

Scheduling: the output projection is backloaded (2/4/6 tiles injected
into blocks qc1/qc2/qc3) because qc3's exp work (~27us) far exceeds
its attention matmul work (~12us) -- the spare projection tiles keep
the PE busy while ACT grinds through the last exps.
"""

import numpy as np

B, T, C, H = 2, 2048, 1024, 16
D = C // H            # 64 head dim
NCORES = 8
NBG = 2               # batch shards
NHG = 4               # head-group shards
HL = H // NHG         # 4 heads per core
DL = HL * D           # 256 local feature dims
NDQ = DL // 128       # 2 partition chunks of local dims
NTB = T // 512        # 4 t-chunks of 512
NKC = T // 128        # 16 key chunks of 128
NQC = T // 512        # 4 query chunks of 512
NTT = T // 128        # 16 t-tiles of 128 (proj / V)

_CACHE = {}


def _build(mode, debug_dump=False):
    """Build + compile the per-core Bass program. mode: causal|full|general."""
    import concourse.bass as bass
    import concourse.bacc as bacc
    import concourse.tile as tile
    import concourse.mybir as mybir

    f32 = mybir.dt.float32
    bf16 = mybir.dt.bfloat16
    Exp = mybir.ActivationFunctionType.Exp
    mult = mybir.AluOpType.mult
    add = mybir.AluOpType.add

    nc = bacc.Bacc(
        "TRN2", target_bir_lowering=False, debug=False, num_devices=NCORES
    )

    xT = nc.dram_tensor("xT", [C, T], bf16, kind="ExternalInput").ap()
    # W pre-permuted on the host so each column group is contiguous per
    # partition (2KB+ DMA lines; the naive [C, 3*DL] layout gives 256B
    # strided segments and ~5us group loads)
    WgQK = nc.dram_tensor(
        "WgQK", [128, 4, 8, 128], bf16, kind="ExternalInput"
    ).ap()
    WgV = nc.dram_tensor("WgV", [128, 8, 256], bf16, kind="ExternalInput").ap()
    # x's first t-chunk, host-permuted contiguous per partition (8KB DMA
    # lines): the startup is DMA-bound, later t-chunks stream fine from xT
    x0g = nc.dram_tensor("x0g", [128, 8, 512], bf16, kind="ExternalInput").ap()
    bqk = nc.dram_tensor("bqk", [128, 2 * NDQ], f32, kind="ExternalInput").ap()
    bv = nc.dram_tensor("bv", [1, DL], f32, kind="ExternalInput").ap()
    Wp = nc.dram_tensor("Wp", [DL, C], bf16, kind="ExternalInput").ap()
    maskT = None
    if mode == "general":
        maskT = nc.dram_tensor("maskT", [T, T], bf16, kind="ExternalInput").ap()
    yp = nc.dram_tensor("yp", [T, C], bf16, kind="ExternalOutput").ap()
    dbg = {}
    if debug_dump:
        for nm, shp, dt in [
            ("ot_d", [128, NDQ, T], bf16),
        ]:
            dbg[nm] = nc.dram_tensor(nm, shp, dt, kind="ExternalOutput").ap()

    with tile.TileContext(nc) as tc:
        with (
            tc.tile_pool(name="singles", bufs=1) as singles,
            tc.tile_pool(name="xin", bufs=2) as xin,
            tc.tile_pool(name="ptiles", bufs=8) as ptiles,
            tc.tile_pool(name="small", bufs=4) as small,
            tc.tile_pool(name="outp", bufs=4) as outp,
            tc.tile_pool(name="psum", bufs=7, space="PSUM") as psum,
        ):
            def ps512(name):
                # transient accumulators (qk/v/proj): 2-bank rotation
                return psum.tile(
                    [128, 512], f32, name="ps512", tag="ps512", bufs=2
                )

            def ps_ops(name):
                # attention P@V accumulators live in their own 2-bank tag:
                # they are held for a whole head-pair, and sharing a
                # rotation with the transients chained every third
                # injected unit onto a pair-long hold
                return psum.tile(
                    [128, 512], f32, name="ops", tag="ops", bufs=2
                )

            # ---- resident inputs ----
            # W loads by COLUMN GROUP in unit-consumption order (Q0, Q1,
            # K0, K1, V) on the sync queue; x's first t-chunk rides the
            # scalar (ACT) hwdge queue -- idle until the first exp -- so
            # the two dispatch chains run in parallel and the first QK
            # unit's operands land ~2us after the preamble.
            x0 = xin.tile([128, 8, 512], bf16, tag="x0", bufs=1)
            # three parallel dispatch chains: W column groups + first x
            # pairs on sync (both host-permuted contiguous per partition),
            # the rest of x then the small inputs on scalar
            Wq0 = singles.tile([128, 8, 128], bf16, name="Wq0")
            nc.sync.dma_start(out=x0[:, 0:2, :], in_=x0g[:, 0:2, :])
            nc.sync.dma_start(out=Wq0[:, 0:3, :], in_=WgQK[:, 0, 0:3, :])
            nc.sync.dma_start(out=x0[:, 2:4, :], in_=x0g[:, 2:4, :])
            nc.sync.dma_start(out=Wq0[:, 3:8, :], in_=WgQK[:, 0, 3:8, :])
            nc.scalar.dma_start(out=x0[:, 4:6, :], in_=x0g[:, 4:6, :])
            nc.scalar.dma_start(out=x0[:, 6:8, :], in_=x0g[:, 6:8, :])
            bqk_sb = singles.tile([128, 2 * NDQ], f32)
            nc.scalar.dma_start(out=bqk_sb, in_=bqk)
            bv_row = singles.tile([1, DL], f32)
            nc.scalar.dma_start(out=bv_row, in_=bv)
            Wcg = [Wq0]
            for gi in range(1, 4):
                wt = singles.tile([128, 8, 128], bf16, name=f"Wcg{gi}")
                nc.sync.dma_start(out=wt, in_=WgQK[:, gi, :, :])
                Wcg.append(wt)
            WV = singles.tile([128, 8, 256], bf16, name="WV")
            nc.sync.dma_start(out=WV, in_=WgV)

            bv_sb = singles.tile([128, DL], f32)
            nc.gpsimd.partition_broadcast(bv_sb, bv_row)

            # 0/1 indicator for the 128-wide diagonal block: every diag
            # chunk uses the same within-block mask (keep iff c >= p)
            indb = None
            if mode == "causal":
                indb = singles.tile([128, 128], bf16, name="indb")
                nc.vector.memset(indb, 1.0)
                nc.gpsimd.affine_select(
                    out=indb, in_=indb,
                    compare_op=mybir.AluOpType.is_ge,
                    fill=0.0, base=0, pattern=[[1, 128]],
                    channel_multiplier=-1,
                )

            # ones row for the tail's one-row broadcast matmuls
            ones1 = singles.tile([1, 64], bf16, name="ones1")
            nc.vector.memset(ones1, 1.0)

            # ---- resident intermediates ----
            QT = singles.tile([128, NDQ, T], bf16)   # [dim%128, dimchunk, t]
            KT = singles.tile([128, NDQ, T], bf16)
            # V plus a ones-column: each head uses cols [0:D+1], so its
            # softmax denominator lands on PSUM partition 64.
            V1 = singles.tile([128, NKC, HL, D + 1], bf16)
            nc.vector.memset(V1[:, :, :, D : D + 1], 1.0)
            OT = singles.tile([128, NDQ, T], bf16)
            Wp_sb = singles.tile([128, NDQ, C], bf16)

            # ---- phase 1: QKV projections (as interleavable units) ----
            def p1_units(tb, xs):
                """Units for one 512-wide t-chunk of the QKV projection.
                xs(kc) -> [128, 512] AP for contraction chunk kc."""
                units = []
                for s in range(2):  # 0=Q, 1=K
                    for dq in range(NDQ):
                        def qk_u(tb=tb, s=s, dq=dq, xs=xs):
                            ps = ps512("qk")
                            g = s * NDQ + dq
                            for kc in range(8):
                                nc.tensor.matmul(
                                    ps,
                                    lhsT=Wcg[g][:, kc, :],
                                    rhs=xs(kc),
                                    start=(kc == 0),
                                    stop=(kc == 7),
                                )
                            dst = (QT if s == 0 else KT)[
                                :, dq, tb * 512 : (tb + 1) * 512
                            ]
                            nc.vector.tensor_scalar_add(
                                dst, ps,
                                bqk_sb[:, g : g + 1],
                            )
                        units.append(qk_u)
                for t4 in range(4):
                    def v_u(tb=tb, t4=t4, xs=xs):
                        tt = tb * 4 + t4
                        ps = ps512("v")
                        for kc in range(8):
                            nc.tensor.matmul(
                                ps[:, :DL],
                                lhsT=xs(kc)[:, t4 * 128 : (t4 + 1) * 128],
                                rhs=WV[:, kc, :],
                                start=(kc == 0),
                                stop=(kc == 7),
                            )
                        nc.vector.tensor_tensor(
                            out=V1[:, tt, :, 0:D],
                            in0=ps[:, :DL].rearrange("p (h d) -> p h d", d=D),
                            in1=bv_sb.rearrange("p (h d) -> p h d", d=D),
                            op=add,
                        )
                    units.append(v_u)
                return units

            # ---- projection units ----
            def proj_units(qc, tail=False, use_scalar=False):
                units = []
                for t4 in range(4):
                    def u(qc=qc, t4=t4, tail=tail):
                        tt = qc * 4 + t4
                        y_sb = outp.tile([128, C], bf16, name="y_sb")
                        for n in range(2):
                            pp = ps512("proj")
                            for dq in range(NDQ):
                                nc.tensor.matmul(
                                    pp,
                                    lhsT=OT[:, dq, tt * 128 : (tt + 1) * 128],
                                    rhs=Wp_sb[:, dq, n * 512 : (n + 1) * 512],
                                    start=(dq == 0),
                                    stop=(dq == NDQ - 1),
                                )
                            dst = y_sb[:, n * 512 : (n + 1) * 512]
                            if (tail or use_scalar) and n == 1:
                                nc.scalar.copy(dst, pp)
                            else:
                                nc.vector.tensor_copy(dst, pp)
                        if tail:
                            for n in range(2):
                                nc.sync.dma_start(
                                    out=yp[tt * 128 : (tt + 1) * 128,
                                           n * 512 : (n + 1) * 512],
                                    in_=y_sb[:, n * 512 : (n + 1) * 512],
                                )
                        else:
                            nc.sync.dma_start(
                                out=yp[tt * 128 : (tt + 1) * 128, :], in_=y_sb
                            )
                    units.append(u)
                return units

            # tail-only split projection: dq=0 accumulation pre-started
            # while the last pair's reciprocal/broadcast chain runs
            pp_state = {}

            def proj_head(tt, big=False, use_ops=False):
                if big:
                    tb = psum.tile(
                        [128, 2, 512], f32, name="ps1024", tag="ps1024",
                        bufs=2,
                    )
                for n in range(2):
                    pp = tb[:, n, :] if big else (
                        ps_ops("projA") if use_ops else ps512("projA")
                    )
                    nc.tensor.matmul(
                        pp,
                        lhsT=OT[:, 0, tt * 128 : (tt + 1) * 128],
                        rhs=Wp_sb[:, 0, n * 512 : (n + 1) * 512],
                        start=True,
                        stop=False,
                    )
                    pp_state[(tt, n)] = pp

            def proj_finish(tt):
                y_sb = outp.tile([128, C], bf16, name="y_sb")
                for n in range(2):
                    pp = pp_state.pop((tt, n))
                    nc.tensor.matmul(
                        pp,
                        lhsT=OT[:, 1, tt * 128 : (tt + 1) * 128],
                        rhs=Wp_sb[:, 1, n * 512 : (n + 1) * 512],
                        start=False,
                        stop=True,
                    )
                    dst = y_sb[:, n * 512 : (n + 1) * 512]
                    if n == 1:
                        nc.scalar.copy(dst, pp)
                    else:
                        nc.vector.tensor_copy(dst, pp)
                for n in range(2):
                    nc.sync.dma_start(
                        out=yp[tt * 128 : (tt + 1) * 128,
                               n * 512 : (n + 1) * 512],
                        in_=y_sb[:, n * 512 : (n + 1) * 512],
                    )

            norm_state = {}

            def attn_units(qc, hp, nkc, m_sb):
                """One head-pair's attention over all k-chunks; softmax
                denominators ride V's ones-column onto PSUM partition 64."""
                state = {}
                last = qc == NQC - 1 and hp == NDQ - 1

                def lo_of(kc):
                    if mode == "causal" and kc >= 4 * qc:
                        return 128 * (kc - 4 * qc)
                    return 0

                def emit_mm1(j):
                    lo = lo_of(j)
                    stp = psum.tile(
                        [128, 2, 512], f32, name="ps1024", tag="ps1024", bufs=2
                    )
                    for hh in range(2):
                        off = 64 * hh
                        nc.tensor.matmul(
                            stp[:, hh, lo:],
                            lhsT=KT[off : off + 64, hp, j * 128 : (j + 1) * 128],
                            rhs=QT[
                                off : off + 64, hp,
                                qc * 512 + lo : (qc + 1) * 512,
                            ],
                            start=True,
                            stop=True,
                        )
                    state.setdefault("st", {})[j] = stp

                def consume(kc):
                    if kc == 0:
                        state["ops"] = [ps_ops("o"), ps_ops("o")]
                    ops = state["ops"]
                    stp = state["st"].pop(kc)
                    lo = lo_of(kc)
                    diag = mode == "causal" and kc >= 4 * qc
                    p2 = ptiles.tile([128, 2, 512], bf16, tag="p")
                    if kc == 0:
                        # split the first exp so the first P@V starts half
                        # an activation earlier
                        nc.scalar.activation(
                            p2[:, :, lo:256], stp[:, :, lo:256], Exp
                        )
                        nc.scalar.activation(
                            p2[:, :, 256:], stp[:, :, 256:], Exp
                        )
                    else:
                        nc.scalar.activation(p2[:, :, lo:], stp[:, :, lo:], Exp)
                    if diag:
                        # zero the upper triangle of the 128-wide diagonal
                        # block (only this block straddles the mask)
                        ind2 = bass.AP(
                            tensor=indb.tensor, offset=indb.offset,
                            ap=[indb.ap[0], [0, 2], indb.ap[1]],
                        )
                        nc.vector.tensor_tensor(
                            out=p2[:, :, lo : lo + 128],
                            in0=p2[:, :, lo : lo + 128],
                            in1=ind2,
                            op=mult,
                        )
                    elif mode == "general":
                        base = m_sb[:, kc, :]
                        msk2 = bass.AP(
                            tensor=base.tensor,
                            offset=base.offset,
                            ap=[base.ap[0], [0, 2], base.ap[1]],
                        )
                        nc.vector.tensor_tensor(
                            out=p2, in0=p2, in1=msk2, op=mult
                        )
                    pieces = [(lo, 256), (256, 512)] if kc == 0 else [(lo, 512)]
                    for hh in range(2):
                        h = hp * 2 + hh
                        for pi, (c0, c1) in enumerate(pieces):
                            nc.tensor.matmul(
                                ops[hh][: D + 1, c0:c1],
                                lhsT=V1[:, kc, h, : D + 1],
                                rhs=p2[:, hh, c0:c1],
                                start=(kc == 0 and pi == 0),
                                stop=(kc == nkc - 1 and pi == len(pieces) - 1),
                            )
                    if state["emitted"] < nkc:
                        emit_mm1(state["emitted"])
                        state["emitted"] += 1

                def prologue():
                    state["emitted"] = min(2, nkc)  # lookahead 1
                    for j in range(state["emitted"]):
                        emit_mm1(j)

                def stash():
                    # OT casts (unnormalized) + denominator rows pulled to
                    # partition 0 by plain -64-shifted vector copies (no
                    # DMA: its ~4us SWDGE latency parked the reciprocal on
                    # the in-order vector queue and froze it)
                    ops = state["ops"]
                    qs = slice(qc * 512, (qc + 1) * 512)
                    sums = small.tile([1, 2, 512], f32, tag="sums", bufs=2)
                    nc.vector.tensor_copy(sums[0:1, 0, :], ops[0][D : D + 1, :])
                    nc.vector.tensor_copy(sums[0:1, 1, :], ops[1][D : D + 1, :])
                    if last:
                        # reciprocal jumps the vector queue ahead of the
                        # OT casts: it gates the whole drain chain
                        rcp = small.tile([1, 2, 512], f32, tag="rcpf", bufs=2)
                        nc.vector.reciprocal_approx_fast(out=rcp, in_=sums)
                        rcpb = small.tile([1, 2, 512], bf16, tag="rcpb", bufs=2)
                        nc.vector.tensor_copy(rcpb[0:1, 0, :], rcp[0:1, 0, :])
                        nc.scalar.copy(rcpb[0:1, 1, :], rcp[0:1, 1, :])
                        state["rcpb"] = rcpb
                        nc.scalar.copy(OT[0:64, hp, qs], ops[0][0:D, :])
                        nc.vector.tensor_copy(OT[64:128, hp, qs], ops[1][0:D, :])
                    else:
                        nc.vector.tensor_copy(OT[0:64, hp, qs], ops[0][0:D, :])
                        nc.vector.tensor_copy(OT[64:128, hp, qs], ops[1][0:D, :])
                    state["sums"] = sums

                def norm_a():
                    # reciprocal at partition 0 (broadcast + scale stay
                    # deferred in norm_b); the tail pair already did it
                    # inside stash
                    sums = state.pop("sums")
                    if last:
                        norm_state[(qc, hp)] = state.pop("rcpb")
                    else:
                        rcp = small.tile([1, 2, 512], f32, tag="rcpf", bufs=2)
                        nc.vector.reciprocal_approx_fast(out=rcp, in_=sums)
                        norm_state[(qc, hp)] = rcp

                units = [prologue]
                for kc in range(nkc):
                    units.append(lambda kc=kc: consume(kc))
                units.append(stash)
                units.append(norm_a)
                return units

            def norm_b(qc, hp):
                # broadcast the partition-0 reciprocal rows (gpsimd) and
                # scale OT in place: hh=0 and hh=1 both on vector (all
                # SBUF -- pool can't touch PSUM and its TTs are slow)
                rcp = norm_state.pop((qc, hp))
                qs = slice(qc * 512, (qc + 1) * 512)
                rb0 = small.tile([128, 512], f32, tag="rb0", bufs=2)
                nc.gpsimd.partition_broadcast(
                    rb0[0:64, :], rcp[0:1, 0, :], channels=64
                )
                rb1 = small.tile([128, 512], f32, tag="rb1", bufs=2)
                nc.gpsimd.partition_broadcast(
                    rb1, rcp[0:1, 1, :], channels=128
                )
                nc.vector.tensor_tensor(
                    out=OT[0:64, hp, qs], in0=OT[0:64, hp, qs],
                    in1=rb0[0:64, :], op=mult,
                )
                nc.vector.tensor_tensor(
                    out=OT[64:128, hp, qs], in0=OT[64:128, hp, qs],
                    in1=rb1[64:128, :], op=mult,
                )

            def norm_tail(qc, hp):
                # tail: broadcast via two one-row PE matmuls (contraction
                # 1, all operands at partition 0) so the gpsimd queue
                # never gates the drain; one full-width scale on vector
                rcpb = norm_state.pop((qc, hp))
                qs = slice(qc * 512, (qc + 1) * 512)
                bb = psum.tile(
                    [128, 2, 512], f32, name="ps1024", tag="ps1024", bufs=2
                )[:, 0, :]
                nc.tensor.matmul(
                    bb[0:64, :], lhsT=ones1, rhs=rcpb[0:1, 0, :],
                    start=True, stop=True,
                )
                nc.tensor.matmul(
                    bb[64:128, :], lhsT=ones1, rhs=rcpb[0:1, 1, :],
                    start=True, stop=True,
                )
                nc.vector.tensor_tensor(
                    out=OT[:, hp, qs], in0=OT[:, hp, qs], in1=bb, op=mult
                )

            # ---- schedule: staircase interleave ----
            # attn(qc) needs phase-1 chunks tb <= qc only, so phase-1(tb+1)
            # and proj(qc-1) units are injected between attention units to
            # keep the PE FIFO fed while ACT paces the exp chain.
            for u in p1_units(0, lambda kc: x0[:, kc, :]):
                u()
            nc.scalar.dma_start(
                out=Wp_sb, in_=Wp.rearrange("(dq p) n -> p dq n", p=128)
            )
            for qc in range(NQC):
                nkc = 4 * qc + 4 if mode == "causal" else NKC
                m_sb = None
                if mode == "general":
                    m_sb = xin.tile([128, NKC, 512], bf16, tag="mask", bufs=1)
                    nc.sync.dma_start(
                        out=m_sb,
                        in_=maskT.rearrange("(kc p) q -> p kc q", p=128)[
                            :, :, qc * 512 : (qc + 1) * 512
                        ],
                    )
                inj_early = []
                if qc + 1 < NTB:
                    x_next = xin.tile(
                        [128, 8, 512], bf16, tag="x_sb", name="x_sb"
                    )
                    def dma_u(tb=qc + 1, x_sb=x_next):
                        xr = xT.rearrange("(kc p) t -> p kc t", p=128)[
                            :, :, tb * 512 : (tb + 1) * 512
                        ]
                        for k2 in range(4):
                            nc.sync.dma_start(
                                out=x_sb[:, 2 * k2 : 2 * k2 + 2, :],
                                in_=xr[:, 2 * k2 : 2 * k2 + 2, :],
                            )
                    inj_early.append(dma_u)
                    inj_early += p1_units(
                        qc + 1, lambda kc, x_sb=x_next: x_sb[:, kc, :]
                    )
                # backload the output projection: qc3 is ACT-bound (32
                # full-width exps vs ~12us of attention matmuls), so spare
                # proj tiles are deferred there to keep the PE fed
                if qc == 1:
                    inj_late = proj_units(0)[0:2]
                elif qc == 2:
                    inj_late = proj_units(0)[2:4] + proj_units(1)[0:2]
                elif qc == 3:
                    inj_late = proj_units(1)[2:4] + proj_units(2)
                else:
                    inj_late = []
                hp_units = []
                for hp in range(NDQ):
                    hp_units.append(attn_units(qc, hp, nkc, m_sb))
                # deferred norm: prior head-pair's broadcast+scale runs a
                # few units into the next block, when its reciprocal and
                # stash are done
                if qc > 0:
                    hp_units[0].insert(
                        3, lambda qc=qc: norm_b(qc - 1, 1)
                    )
                hp_units[1].insert(3, lambda qc=qc: norm_b(qc, 0))
                if qc == NQC - 1:
                    # pre-start the dq=0 half of three projection tiles:
                    # these matmuls fill the PE while the last pair's
                    # reciprocal chain runs off-engine
                    hp_units[1].append(lambda: proj_head(NTT - 4, big=True))
                    hp_units[1].append(lambda: proj_head(NTT - 3))
                    hp_units[1].append(lambda: proj_head(NTT - 2, use_ops=True))
                    hp_units[1].append(lambda qc=qc: norm_tail(qc, 1))
                main = hp_units[0] + hp_units[1]
                # the appended tail units (proj_head x3 + norm_tail) hold
                # all ps512 slots; no injections may land after them
                ntail = 4 if qc == NQC - 1 else 0
                half = (len(main) - ntail + 1) // 2
                mid = len(main) - ntail
                for part, inj in (
                    (main[:half], inj_early),
                    (main[half:mid], inj_late),
                ):
                    k, m, j = len(part), len(inj), 0
                    for i, u in enumerate(part):
                        u()
                        take = (i + 1) * m // k - i * m // k
                        for _ in range(take):
                            inj[j]()
                            j += 1
                for u in main[mid:]:
                    u()
            proj_finish(NTT - 4)
            proj_finish(NTT - 3)
            proj_finish(NTT - 2)
            for u in proj_units(NQC - 1, tail=True)[3:]:
                u()

            if debug_dump:
                nc.sync.dma_start(out=dbg["ot_d"], in_=OT)

    nc.compile()
    return nc


def _host_prep(x, prefix_causal_mask, W_attn, b_attn, W_proj):
    """Split full inputs into 8 per-core input maps; detect mask mode."""
    scale = 1.0 / np.sqrt(np.float32(D))
    mask = np.asarray(prefix_causal_mask)
    if mask.all():
        mode = "full"
    else:
        tri = np.tril(np.ones((T, T), dtype=bool))
        if all(np.array_equal(mask[b], tri) for b in range(B)):
            mode = "causal"
        else:
            mode = "general"

    import ml_dtypes

    bf16 = ml_dtypes.bfloat16
    x = np.asarray(x, dtype=np.float32)
    W_attn = np.asarray(W_attn, dtype=np.float32)
    b_attn = np.asarray(b_attn, dtype=np.float32)
    W_proj = np.asarray(W_proj, dtype=np.float32)

    in_maps = []
    for core in range(NCORES):
        b = core // NHG
        hg = core % NHG
        lo = hg * DL
        hi = lo + DL
        xT = np.ascontiguousarray(x[b].T)  # [C, T]
        Wq = W_attn[:, lo:hi] * scale
        Wk = W_attn[:, C + lo : C + hi]
        Wv = W_attn[:, 2 * C + lo : 2 * C + hi]
        Wl = np.concatenate([Wq, Wk], axis=1)  # [C, 512]
        # group-contiguous permutation: WgQK[p, g, kc, n] = Wl[kc*128+p, g*128+n]
        WgQK = np.ascontiguousarray(
            Wl.reshape(8, 128, 4, 128).transpose(1, 2, 0, 3)
        )
        WgV = np.ascontiguousarray(Wv.reshape(8, 128, 256).transpose(1, 0, 2))
        x0g = np.ascontiguousarray(
            xT[:, 0:512].reshape(8, 128, 512).transpose(1, 0, 2)
        )
        bq = b_attn[lo:hi] * scale
        bk = b_attn[C + lo : C + hi]
        # bias per partition for Q,K chunks: cols = [q0, q1, k0, k1]
        bqk = np.stack(
            [bq[0:128], bq[128:256], bk[0:128], bk[128:256]], axis=1
        ).astype(np.float32)
        bv = np.ascontiguousarray(
            b_attn[2 * C + lo : 2 * C + hi][None, :]
        ).astype(np.float32)
        Wp = np.ascontiguousarray(W_proj[lo:hi, :])
        im = {
            "xT": xT.astype(bf16),
            "WgQK": WgQK.astype(bf16),
            "WgV": WgV.astype(bf16),
            "x0g": x0g.astype(bf16),
            "bqk": np.ascontiguousarray(bqk),
            "bv": bv,
            "Wp": Wp.astype(bf16),
        }
        if mode == "general":
            im["maskT"] = np.ascontiguousarray(mask[b].T).astype(bf16)
        in_maps.append(im)
    return mode, in_maps


def _get_program(mode):
    if mode not in _CACHE:
        _CACHE[mode] = _build(mode)
    return _CACHE[mode]


def _run(inputs, trace=False):
    """Returns (full_output [B,T,C], BassKernelResults)."""
    from concourse import bass_utils

    mode, in_maps = _host_prep(
        inputs["x"],
        inputs["prefix_causal_mask"],
        inputs["W_attn"],
        inputs["b_attn"],
        inputs["W_proj"],
    )
    nc = _get_program(mode)
    res = bass_utils.run_bass_kernel_spmd(
        nc, in_maps, core_ids=list(range(NCORES)), trace=trace
    )
    b_proj = np.asarray(inputs["b_proj"], dtype=np.float32)
    y = np.zeros((B, T, C), dtype=np.float32)
    for core in range(NCORES):
        y[core // NHG] += np.asarray(res.results[core]["yp"], dtype=np.float32)
    y += b_proj[None, None, :]
    return y, res


def kernel(**inputs):
    y, _ = _run(inputs, trace=False)
    return y
